# revision 13
# baseline (speedup 1.0000x reference)
"""Trainium2 Bass kernel for a 12-head causal attention block (GPT-2 style).

Problem: x:[4,2048,768] -> qkv = x@W_attn+b_attn, causal softmax attention
(12 heads, d=64), out @ W_proj + b_proj.

Sharding over 8 NeuronCores: core c handles batch b=c//2 (data parallel) and
head-group hg=c%2 (6 heads, tensor parallel on the qkv columns / proj rows).
Each core returns a partial projection output; the host sums the two
head-group partials per batch and adds b_proj.

Per-core dataflow (all matmuls in float32r: full speed, ~1e-3 rel err):
  - x [2048,768] is PE-transposed to xT (emb on partitions).
  - qkvT = W-tiles.T @ xT  -> qT,kT per head-pair [128,2048]; v is computed
    non-transposed (v = xT-tiles.T @ Wv) since P@V needs V with seq on
    partitions.  b_attn folded in (per-partition add for q/k, rank-1 matmul
    for v).
  - scores S^T[k,q] per 128k x 512q block: lhsT=kT[d=64 rows], rhs=qT.  The
    two heads of a pair run row-packed (tile_position (0,0)/(64,0)) writing
    adjacent PSUM banks, so one ACT exp call covers both heads.
  - causal: upper-triangle blocks are skipped entirely; the diagonal-crossing
    128x128 triangle is zeroed post-exp with gpsimd affine_select.  Softmax
    needs no max-subtraction here (|scores/8| < ~4, exp is safe in fp32).
  - P@V and the softmax denominators accumulate in PSUM over k-tiles:
    AV col-packed per head pair; the denominator matmul uses an all-ones
    [128,64] lhsT so the sums land already broadcast across 64 partitions;
    DVE reciprocal + multiply then writes normalized attn-out^T to SBUF.
  - proj: y[128q,768] accumulated over the 3 head-pair k-tiles, DMA'd out
    straight from PSUM.
"""

import os
import ml_dtypes
import numpy as np

N_HEAD = 12
N_EMBD = 768
HEAD_DIM = 64
B, S = 4, 2048
N_CORES = 8
HG_HEADS = 6            # heads per core (3 pairs)
HG_DIM = HG_HEADS * HEAD_DIM   # 384
QKV_W = 3 * HG_DIM      # 1152 qkv columns per core
N_PAIRS = 3
ST = S // 128           # 16 seq tiles of 128
NG = S // 512           # 4 seq groups of 512

# last run's BassKernelResults (test.py reads this for HW timing / traces)
LAST_RESULTS = None
_PROGRAM = None


def _build_program():
    """Build (once) the SPMD Bass program run identically on all 8 cores."""
    import concourse.bacc as bacc
    import concourse.tile as tile
    from concourse import mybir, masks

    F32R = mybir.dt.float32r
    F32 = mybir.dt.float32
    BF16 = mybir.dt.bfloat16
    AF = mybir.ActivationFunctionType

    nc = bacc.Bacc(None, target_bir_lowering=False)
    x_d = nc.declare_dram_parameter("x", [S, N_EMBD], F32, isOutput=False)
    wqkv_d = nc.declare_dram_parameter("w_qkv", [N_EMBD, QKV_W], F32R, isOutput=False)
    bqk_d = nc.declare_dram_parameter("b_qk", [768], F32, isOutput=False)
    bv_d = nc.declare_dram_parameter("b_v", [HG_DIM], F32R, isOutput=False)
    wproj_d = nc.declare_dram_parameter("w_proj", [HG_DIM, N_EMBD], F32R, isOutput=False)
    ones_d = nc.declare_dram_parameter("ones", [1, 128], F32R, isOutput=False)
    onesbf_d = nc.declare_dram_parameter("ones_bf", [128, 64], BF16, isOutput=False)
    y_d = nc.declare_dram_parameter("y", [S, N_EMBD], F32, isOutput=True)

    with tile.TileContext(nc) as tc:
        from contextlib import ExitStack

        with ExitStack() as outer:
            consts = outer.enter_context(tc.tile_pool(name="consts", bufs=1))
            ident = consts.tile([128, 128], F32)
            masks.make_identity(nc, ident[:])
            ones_row = consts.tile([1, 128], F32R)    # v-bias rank-1 lhsT
            nc.sync.dma_start(out=ones_row[:], in_=ones_d[:])
            ones_bf = consts.tile([128, 64], BF16)    # sums-broadcast lhsT
            nc.sync.dma_start(out=ones_bf[:], in_=onesbf_d[:])
            bias_qk = consts.tile([128, 6], F32)      # col m: b_qk[128m:128m+128]
            nc.sync.dma_start(
                out=bias_qk[:], in_=bqk_d[0:768].rearrange("(m p) -> p m", p=128)
            )
            bias_v = consts.tile([1, HG_DIM], F32R)
            nc.sync.dma_start(
                out=bias_v[:], in_=bv_d[0:HG_DIM].rearrange("(o v) -> o v", o=1)
            )

            # ---- persistent activations/weights in SBUF ----
            big = outer.enter_context(tc.tile_pool(name="big", bufs=1))
            xT = big.tile([128, 6 * S], F32R)      # [emb-part, k-tile*2048+seq]
            qkT = big.tile([128, 6 * S], BF16)     # m=0..2 qT pairs, m=3..5 kT pairs
            q_odd = big.tile([64, N_PAIRS * S], BF16)  # odd heads shifted to base 0
            k_odd = big.tile([64, N_PAIRS * S], BF16)
            v_all = big.tile([128, ST * HG_DIM], BF16)  # [seq-part, t*384 + 64h+d]
            attnT = big.tile([128, N_PAIRS * S], F32R)  # [pair d, pair*2048+seq]
            w_proj = big.tile([128, N_PAIRS * N_EMBD], F32R)
            for p in range(N_PAIRS):
                nc.sync.dma_start(
                    out=w_proj[:, p * N_EMBD:(p + 1) * N_EMBD],
                    in_=wproj_d[p * 128:(p + 1) * 128, :],
                )

            # ---- phase A: load x tiles + PE-transpose into xT ----
            with tc.tile_pool(name="xload", bufs=3) as xload, \
                 tc.tile_pool(name="tps", bufs=2, space="PSUM") as tps:
                xT_v = xT[:].rearrange("p (k s) -> p k s", k=6)
                for t in range(ST):
                    xs = xload.tile([128, N_EMBD], F32)
                    nc.sync.dma_start(out=xs[:], in_=x_d[t * 128:(t + 1) * 128, :])
                    tp = tps.tile([128, N_EMBD], F32)
                    for k in range(6):
                        nc.tensor.transpose(
                            tp[:, k * 128:(k + 1) * 128],
                            xs[:, k * 128:(k + 1) * 128],
                            ident[:],
                        )
                    nc.vector.tensor_copy(
                        xT_v[:, :, t * 128:(t + 1) * 128],
                        tp[:].rearrange("p (k s) -> p k s", k=6),
                    )

            # ---- phase B: qkv projections ----
            with tc.tile_pool(name="wqkv", bufs=1) as wq_pool, \
                 tc.tile_pool(name="qkps", bufs=4, space="PSUM") as qkps, \
                 tc.tile_pool(name="vps", bufs=2, space="PSUM") as vps:
                w_all = wq_pool.tile([128, 6 * QKV_W], F32R)
                for k in range(6):
                    nc.sync.dma_start(
                        out=w_all[:, k * QKV_W:(k + 1) * QKV_W],
                        in_=wqkv_d[k * 128:(k + 1) * 128, :],
                    )
                # q/k: transposed layout -> qkT
                for m in range(6):
                    for g in range(NG):
                        ps = qkps.tile([128, 512], F32)
                        for k in range(6):
                            nc.tensor.matmul(
                                ps[:],
                                w_all[:, k * QKV_W + m * 128:k * QKV_W + (m + 1) * 128],
                                xT[:, k * S + g * 512:k * S + g * 512 + 512],
                                start=(k == 0), stop=(k == 5),
                            )
                        nc.vector.tensor_scalar_add(
                            qkT[:, m * S + g * 512:m * S + g * 512 + 512],
                            ps[:], bias_qk[:, m:m + 1],
                        )
                # v: natural [seq, d] layout
                v_v = v_all[:].rearrange("p (t d) -> p t d", t=ST)
                for t in range(ST):
                    ps = vps.tile([128, HG_DIM], F32)
                    for k in range(6):
                        nc.tensor.matmul(
                            ps[:],
                            xT[:, k * S + t * 128:k * S + (t + 1) * 128],
                            w_all[:, k * QKV_W + 768:k * QKV_W + QKV_W],
                            start=(k == 0), stop=False,
                        )
                    nc.tensor.matmul(   # += ones^T[1,128].T @ bias_v[1,384]
                        ps[:], ones_row[:], bias_v[:], start=False, stop=True,
                    )
                    nc.vector.tensor_copy(v_v[:, t, :], ps[:])

            # ---- phase C: causal attention, one head-pair at a time ----
            # odd heads' qT/kT shifted to partition base 0 (SBUF->SBUF DMA);
            # a matmul lhsT/rhs at base partition 64 crashes at runtime.
            for pair in range(N_PAIRS):
                nc.sync.dma_start(
                    out=q_odd[:, pair * S:(pair + 1) * S],
                    in_=qkT[64:128, pair * S:(pair + 1) * S])
                nc.sync.dma_start(
                    out=k_odd[:, pair * S:(pair + 1) * S],
                    in_=qkT[64:128, (3 + pair) * S:(4 + pair) * S])
            with tc.tile_pool(name="stps", bufs=3, space="PSUM") as stps, \
                 tc.tile_pool(name="avps", bufs=1, space="PSUM") as avps, \
                 tc.tile_pool(name="smps", bufs=1, space="PSUM") as smps, \
                 tc.tile_pool(name="ptp", bufs=3) as ptp, \
                 tc.tile_pool(name="rcp", bufs=2) as rcp:
                for pair in range(N_PAIRS):
                    q0 = pair * S          # qT pair tile offset in qkT
                    k0 = (3 + pair) * S    # kT pair tile offset
                    for g in range(NG):
                        av = avps.tile([128, 512], F32)
                        sm = smps.tile([128, 512], F32)
                        njt = 4 * g + 4
                        for j in range(njt):
                            diag_r = j - 4 * g   # >=0 on diagonal-crossing tiles
                            c0 = 128 * diag_r if diag_r >= 0 else 0
                            st = stps.tile([128, 1024], F32)   # h1 | h2
                            pt = ptp.tile([128, 1024], BF16)
                            nc.tensor.matmul(
                                st[:, c0:512],
                                qkT[0:64, k0 + j * 128:k0 + (j + 1) * 128],
                                qkT[0:64, q0 + g * 512 + c0:q0 + (g + 1) * 512],
                                start=True, stop=True,
                            )
                            nc.tensor.matmul(
                                st[:, 512 + c0:1024],
                                k_odd[:, q0 + j * 128:q0 + (j + 1) * 128],
                                q_odd[:, q0 + g * 512 + c0:q0 + (g + 1) * 512],
                                start=True, stop=True,
                            )
                            # exp(S/8) over both heads' valid columns
                            nc.scalar.activation(
                                pt[:, c0:1024], st[:, c0:1024], AF.Exp,
                                bias=0.0, scale=0.125,
                            )
                            if diag_r >= 0:
                                # zero the strictly-lower (k>q) triangle
                                for h in range(2):
                                    nc.gpsimd.affine_select(
                                        out=pt[:, h * 512 + c0:h * 512 + c0 + 128],
                                        in_=pt[:, h * 512 + c0:h * 512 + c0 + 128],
                                        compare_op=mybir.AluOpType.is_ge,
                                        fill=0.0, base=0,
                                        pattern=[[1, 128]], channel_multiplier=-1,
                                    )
                            first, last = (j == 0), (j == njt - 1)
                            for h in range(2):
                                hv = (2 * pair + h) * HEAD_DIM
                                nc.tensor.matmul(   # attn-out^T accumulate
                                    av[64 * h:64 * h + 64, c0:512],
                                    v_all[:, j * HG_DIM + hv:j * HG_DIM + hv + 64],
                                    pt[:, h * 512 + c0:(h + 1) * 512],
                                    start=first, stop=last,
                                    tile_position=(0, 64 * h),
                                )
                            for h in range(2):
                                nc.tensor.matmul(   # denominators, pre-broadcast
                                    sm[64 * h:64 * h + 64, c0:512],
                                    ones_bf[:],
                                    pt[:, h * 512 + c0:(h + 1) * 512],
                                    start=first, stop=last,
                                    tile_position=(0, 64 * h),
                                )
                        rc = rcp.tile([128, 512], F32)
                        nc.vector.reciprocal(rc[:], sm[:])
                        nc.vector.tensor_mul(
                            attnT[:, pair * S + g * 512:pair * S + (g + 1) * 512],
                            av[:], rc[:],
                        )

            # ---- phase D: output projection (partial; host adds b_proj) ----
            with tc.tile_pool(name="yps", bufs=3, space="PSUM") as yps, \
                 tc.tile_pool(name="ystage", bufs=3) as ystage:
                for t in range(ST):
                    ps = yps.tile([128, N_EMBD], F32)
                    for p in range(N_PAIRS):
                        for h0, hw in ((0, 512), (512, 256)):
                            nc.tensor.matmul(
                                ps[:, h0:h0 + hw],
                                attnT[:, p * S + t * 128:p * S + (t + 1) * 128],
                                w_proj[:, p * N_EMBD + h0:p * N_EMBD + h0 + hw],
                                start=(p == 0), stop=(p == N_PAIRS - 1),
                            )
                    ys = ystage.tile([128, N_EMBD], F32)
                    nc.vector.tensor_copy(ys[:], ps[:])
                    nc.sync.dma_start(out=y_d[t * 128:(t + 1) * 128, :], in_=ys[:])

    nc.compile()
    return nc


def _numpy_fallback(x, mask, W_attn, b_attn, W_proj, b_proj):
    qkv = x @ W_attn + b_attn
    q, k, v = np.split(qkv, 3, axis=-1)

    def heads(t):
        return t.reshape(B, S, N_HEAD, HEAD_DIM).transpose(0, 2, 1, 3)

    q, k, v = heads(q), heads(k), heads(v)
    attn = np.einsum("bhqd,bhkd->bhqk", q, k) / np.sqrt(np.float32(HEAD_DIM))
    attn = attn + mask * (-1e9)
    attn = attn - attn.max(axis=-1, keepdims=True)
    attn = np.exp(attn)
    attn = attn / attn.sum(axis=-1, keepdims=True)
    out = np.einsum("bhqk,bhkd->bhqd", attn, v)
    out = out.transpose(0, 2, 1, 3).reshape(B, S, N_EMBD)
    return (out @ W_proj + b_proj).astype(np.float32)


def kernel(x, mask, W_attn, b_attn, W_proj, b_proj):
    global LAST_RESULTS, _PROGRAM
    x = np.asarray(x, dtype=np.float32)
    mask = np.asarray(mask, dtype=np.float32)
    W_attn = np.asarray(W_attn, dtype=np.float32)
    b_attn = np.asarray(b_attn, dtype=np.float32)
    W_proj = np.asarray(W_proj, dtype=np.float32)
    b_proj = np.asarray(b_proj, dtype=np.float32)

    # the kernel exploits causal structure; verify the mask actually is causal
    causal = 1.0 - np.tril(np.ones((S, S), dtype=np.float32))
    if mask.shape != (1, 1, S, S) or not np.array_equal(mask[0, 0], causal):
        return _numpy_fallback(x, mask, W_attn, b_attn, W_proj, b_proj)

    from concourse.bass_utils import run_bass_kernel_spmd

    if _PROGRAM is None:
        _PROGRAM = _build_program()

    in_maps = make_in_maps(x, W_attn, b_attn, W_proj)

    trace = bool(int(os.environ.get("ATTN_KERNEL_TRACE", "0")))
    res = run_bass_kernel_spmd(_PROGRAM, in_maps, list(range(N_CORES)), trace=trace)
    LAST_RESULTS = res

    y = np.zeros((B, S, N_EMBD), dtype=np.float32)
    for c in range(N_CORES):
        y[c // 2] += res.results[c]["y"]
    y += b_proj
    return y


def make_in_maps(x, W_attn, b_attn, W_proj):
    in_maps = []
    for c in range(N_CORES):
        b, hg = divmod(c, 2)
        o = HG_DIM * hg
        in_maps.append({
            "x": np.ascontiguousarray(x[b]),
            "w_qkv": np.ascontiguousarray(np.concatenate(
                [W_attn[:, o:o + HG_DIM],
                 W_attn[:, 768 + o:768 + o + HG_DIM],
                 W_attn[:, 1536 + o:1536 + o + HG_DIM]], axis=1)),
            "b_qk": np.ascontiguousarray(np.concatenate(
                [b_attn[o:o + HG_DIM], b_attn[768 + o:768 + o + HG_DIM]])),
            "b_v": np.ascontiguousarray(b_attn[1536 + o:1536 + o + HG_DIM]),
            "w_proj": np.ascontiguousarray(W_proj[o:o + HG_DIM, :]),
            "ones": np.ones((1, 128), dtype=np.float32),
            "ones_bf": np.ones((128, 64), dtype=ml_dtypes.bfloat16),
        })
    return in_maps


# revision 15
# speedup vs baseline: 99.8779x; 99.8779x over previous
"""Trainium2 Bass kernel for a 12-head causal attention block (GPT-2 style).

Problem: x:[4,2048,768] -> qkv = x@W_attn+b_attn, causal softmax attention
(12 heads, d=64), out @ W_proj + b_proj.

Sharding over 8 NeuronCores: core c handles batch b=c//2 (data parallel) and
head-group hg=c%2 (6 heads, tensor parallel on the qkv columns / proj rows).
Each core returns a partial projection output; the host sums the two
head-group partials per batch and adds b_proj.

Per-core dataflow (all matmuls in float32r: full speed, ~1e-3 rel err):
  - x [2048,768] is PE-transposed to xT (emb on partitions).
  - qkvT = W-tiles.T @ xT  -> qT,kT per head-pair [128,2048]; v is computed
    non-transposed (v = xT-tiles.T @ Wv) since P@V needs V with seq on
    partitions.  b_attn folded in (per-partition add for q/k, rank-1 matmul
    for v).
  - scores S^T[k,q] per 128k x 512q block: lhsT=kT[d=64 rows], rhs=qT.  The
    two heads of a pair run row-packed (tile_position (0,0)/(64,0)) writing
    adjacent PSUM banks, so one ACT exp call covers both heads.
  - causal: upper-triangle blocks are skipped entirely; the diagonal-crossing
    128x128 triangle is zeroed post-exp with gpsimd affine_select.  Softmax
    needs no max-subtraction here (|scores/8| < ~4, exp is safe in fp32).
  - P@V and the softmax denominators accumulate in PSUM over k-tiles:
    AV col-packed per head pair; the denominator matmul uses an all-ones
    [128,64] lhsT so the sums land already broadcast across 64 partitions;
    DVE reciprocal + multiply then writes normalized attn-out^T to SBUF.
  - proj: y[128q,768] accumulated over the 3 head-pair k-tiles, DMA'd out
    straight from PSUM.
"""

import os
import ml_dtypes
import numpy as np

N_HEAD = 12
N_EMBD = 768
HEAD_DIM = 64
B, S = 4, 2048
N_CORES = 8
HG_HEADS = 6            # heads per core (3 pairs)
HG_DIM = HG_HEADS * HEAD_DIM   # 384
QKV_W = 3 * HG_DIM      # 1152 qkv columns per core
N_PAIRS = 3
ST = S // 128           # 16 seq tiles of 128
NG = S // 512           # 4 seq groups of 512

# last run's BassKernelResults (test.py reads this for HW timing / traces)
LAST_RESULTS = None
_PROGRAM = None


def _build_program(loop_n=None):
    """Build (once) the SPMD Bass program run identically on all 8 cores.

    loop_n: benchmark mode — inputs become internal DRAM tensors (no host
    transfer) and the whole kernel body repeats loop_n times in a hardware
    loop, so per-iteration time can be measured as a slope between two
    loop counts (the axon tunnel's dispatch/transfer jitter cancels).
    """
    import concourse.bacc as bacc
    import concourse.tile as tile
    from concourse import mybir, masks

    F32R = mybir.dt.float32r
    F32 = mybir.dt.float32
    BF16 = mybir.dt.bfloat16
    AF = mybir.ActivationFunctionType

    nc = bacc.Bacc(None, target_bir_lowering=False)
    if loop_n is not None:
        dummy_d = nc.declare_dram_parameter("bench_in", [1, 128], F32, isOutput=False)
        tout_d = nc.declare_dram_parameter("bench_out", [1, 128], F32, isOutput=True)
        x_d = nc.dram_tensor("x", [S, N_EMBD], F32)
        wqkv_d = nc.dram_tensor("w_qkv", [N_EMBD, QKV_W], F32R)
        bqk_d = nc.dram_tensor("b_qk", [768], F32)
        bv_d = nc.dram_tensor("b_v", [HG_DIM], F32R)
        wproj_d = nc.dram_tensor("w_proj", [HG_DIM, N_EMBD], F32R)
        ones_d = nc.dram_tensor("ones", [1, 128], F32R)
        onesbf_d = nc.dram_tensor("ones_bf", [128, 64], BF16)
        y_d = nc.dram_tensor("y", [S, N_EMBD], F32)
    else:
        x_d = nc.declare_dram_parameter("x", [S, N_EMBD], F32, isOutput=False)
        wqkv_d = nc.declare_dram_parameter("w_qkv", [N_EMBD, QKV_W], F32R, isOutput=False)
        bqk_d = nc.declare_dram_parameter("b_qk", [768], F32, isOutput=False)
        bv_d = nc.declare_dram_parameter("b_v", [HG_DIM], F32R, isOutput=False)
        wproj_d = nc.declare_dram_parameter("w_proj", [HG_DIM, N_EMBD], F32R, isOutput=False)
        ones_d = nc.declare_dram_parameter("ones", [1, 128], F32R, isOutput=False)
        onesbf_d = nc.declare_dram_parameter("ones_bf", [128, 64], BF16, isOutput=False)
        y_d = nc.declare_dram_parameter("y", [S, N_EMBD], F32, isOutput=True)

    with tile.TileContext(nc) as tc:
        from contextlib import ExitStack

        with ExitStack() as outer:
            if loop_n is not None:
                outer.enter_context(tc.For_i(0, loop_n, 1))
            consts = outer.enter_context(tc.tile_pool(name="consts", bufs=1))
            ident = consts.tile([128, 128], F32)
            masks.make_identity(nc, ident[:])
            ones_row = consts.tile([1, 128], F32R)    # v-bias rank-1 lhsT
            nc.sync.dma_start(out=ones_row[:], in_=ones_d[:])
            ones_bf = consts.tile([128, 64], BF16)    # sums-broadcast lhsT
            nc.sync.dma_start(out=ones_bf[:], in_=onesbf_d[:])
            bias_qk = consts.tile([128, 6], F32)      # col m: b_qk[128m:128m+128]
            nc.sync.dma_start(
                out=bias_qk[:], in_=bqk_d[0:768].rearrange("(m p) -> p m", p=128)
            )
            bias_v = consts.tile([1, HG_DIM], F32R)
            nc.sync.dma_start(
                out=bias_v[:], in_=bv_d[0:HG_DIM].rearrange("(o v) -> o v", o=1)
            )

            # ---- persistent activations/weights in SBUF ----
            big = outer.enter_context(tc.tile_pool(name="big", bufs=1))
            xT = big.tile([128, 6 * S], F32R)      # [emb-part, k-tile*2048+seq]
            qkT = big.tile([128, 6 * S], BF16)     # m=0..2 qT pairs, m=3..5 kT pairs
            q_odd = big.tile([64, N_PAIRS * S], BF16)  # odd heads shifted to base 0
            k_odd = big.tile([64, N_PAIRS * S], BF16)
            v_all = big.tile([128, ST * HG_DIM], BF16)  # [seq-part, t*384 + 64h+d]
            attnT = big.tile([128, N_PAIRS * S], F32R)  # [pair d, pair*2048+seq]
            w_proj = big.tile([128, N_PAIRS * N_EMBD], F32R)
            for p in range(N_PAIRS):
                nc.sync.dma_start(
                    out=w_proj[:, p * N_EMBD:(p + 1) * N_EMBD],
                    in_=wproj_d[p * 128:(p + 1) * 128, :],
                )

            # ---- phase A: load x tiles + PE-transpose into xT ----
            with tc.tile_pool(name="xload", bufs=3) as xload, \
                 tc.tile_pool(name="tps", bufs=2, space="PSUM") as tps:
                xT_v = xT[:].rearrange("p (k s) -> p k s", k=6)
                for t in range(ST):
                    xs = xload.tile([128, N_EMBD], F32)
                    nc.sync.dma_start(out=xs[:], in_=x_d[t * 128:(t + 1) * 128, :])
                    tp = tps.tile([128, N_EMBD], F32)
                    for k in range(6):
                        nc.tensor.transpose(
                            tp[:, k * 128:(k + 1) * 128],
                            xs[:, k * 128:(k + 1) * 128],
                            ident[:],
                        )
                    nc.vector.tensor_copy(
                        xT_v[:, :, t * 128:(t + 1) * 128],
                        tp[:].rearrange("p (k s) -> p k s", k=6),
                    )

            # ---- phase B: qkv projections ----
            with tc.tile_pool(name="wqkv", bufs=1) as wq_pool, \
                 tc.tile_pool(name="qkps", bufs=4, space="PSUM") as qkps, \
                 tc.tile_pool(name="vps", bufs=2, space="PSUM") as vps:
                w_all = wq_pool.tile([128, 6 * QKV_W], F32R)
                for k in range(6):
                    nc.sync.dma_start(
                        out=w_all[:, k * QKV_W:(k + 1) * QKV_W],
                        in_=wqkv_d[k * 128:(k + 1) * 128, :],
                    )
                # q/k: transposed layout -> qkT
                for m in range(6):
                    for g in range(NG):
                        ps = qkps.tile([128, 512], F32)
                        for k in range(6):
                            nc.tensor.matmul(
                                ps[:],
                                w_all[:, k * QKV_W + m * 128:k * QKV_W + (m + 1) * 128],
                                xT[:, k * S + g * 512:k * S + g * 512 + 512],
                                start=(k == 0), stop=(k == 5),
                            )
                        nc.vector.tensor_scalar_add(
                            qkT[:, m * S + g * 512:m * S + g * 512 + 512],
                            ps[:], bias_qk[:, m:m + 1],
                        )
                # v: natural [seq, d] layout
                v_v = v_all[:].rearrange("p (t d) -> p t d", t=ST)
                for t in range(ST):
                    ps = vps.tile([128, HG_DIM], F32)
                    for k in range(6):
                        nc.tensor.matmul(
                            ps[:],
                            xT[:, k * S + t * 128:k * S + (t + 1) * 128],
                            w_all[:, k * QKV_W + 768:k * QKV_W + QKV_W],
                            start=(k == 0), stop=False,
                        )
                    nc.tensor.matmul(   # += ones^T[1,128].T @ bias_v[1,384]
                        ps[:], ones_row[:], bias_v[:], start=False, stop=True,
                    )
                    nc.vector.tensor_copy(v_v[:, t, :], ps[:])

            # ---- phase C: causal attention, one head-pair at a time ----
            # odd heads' qT/kT shifted to partition base 0 (SBUF->SBUF DMA);
            # a matmul lhsT/rhs at base partition 64 crashes at runtime.
            for pair in range(N_PAIRS):
                nc.sync.dma_start(
                    out=q_odd[:, pair * S:(pair + 1) * S],
                    in_=qkT[64:128, pair * S:(pair + 1) * S])
                nc.sync.dma_start(
                    out=k_odd[:, pair * S:(pair + 1) * S],
                    in_=qkT[64:128, (3 + pair) * S:(4 + pair) * S])
            with tc.tile_pool(name="stps", bufs=3, space="PSUM") as stps, \
                 tc.tile_pool(name="avps", bufs=1, space="PSUM") as avps, \
                 tc.tile_pool(name="smps", bufs=1, space="PSUM") as smps, \
                 tc.tile_pool(name="ptp", bufs=3) as ptp, \
                 tc.tile_pool(name="rcp", bufs=2) as rcp:
                for pair in range(N_PAIRS):
                    q0 = pair * S          # qT pair tile offset in qkT
                    k0 = (3 + pair) * S    # kT pair tile offset
                    for g in range(NG):
                        av = avps.tile([128, 512], F32)
                        sm = smps.tile([128, 512], F32)
                        njt = 4 * g + 4
                        for j in range(njt):
                            diag_r = j - 4 * g   # >=0 on diagonal-crossing tiles
                            c0 = 128 * diag_r if diag_r >= 0 else 0
                            st = stps.tile([128, 1024], F32)   # h1 | h2
                            pt = ptp.tile([128, 1024], BF16)
                            nc.tensor.matmul(
                                st[:, c0:512],
                                qkT[0:64, k0 + j * 128:k0 + (j + 1) * 128],
                                qkT[0:64, q0 + g * 512 + c0:q0 + (g + 1) * 512],
                                start=True, stop=True,
                            )
                            nc.tensor.matmul(
                                st[:, 512 + c0:1024],
                                k_odd[:, q0 + j * 128:q0 + (j + 1) * 128],
                                q_odd[:, q0 + g * 512 + c0:q0 + (g + 1) * 512],
                                start=True, stop=True,
                            )
                            # exp(S/8) over both heads' valid columns
                            nc.scalar.activation(
                                pt[:, c0:1024], st[:, c0:1024], AF.Exp,
                                bias=0.0, scale=0.125,
                            )
                            if diag_r >= 0:
                                # zero the strictly-lower (k>q) triangle
                                for h in range(2):
                                    nc.gpsimd.affine_select(
                                        out=pt[:, h * 512 + c0:h * 512 + c0 + 128],
                                        in_=pt[:, h * 512 + c0:h * 512 + c0 + 128],
                                        compare_op=mybir.AluOpType.is_ge,
                                        fill=0.0, base=0,
                                        pattern=[[1, 128]], channel_multiplier=-1,
                                    )
                            first, last = (j == 0), (j == njt - 1)
                            for h in range(2):
                                hv = (2 * pair + h) * HEAD_DIM
                                nc.tensor.matmul(   # attn-out^T accumulate
                                    av[64 * h:64 * h + 64, c0:512],
                                    v_all[:, j * HG_DIM + hv:j * HG_DIM + hv + 64],
                                    pt[:, h * 512 + c0:(h + 1) * 512],
                                    start=first, stop=last,
                                    tile_position=(0, 64 * h),
                                )
                            for h in range(2):
                                nc.tensor.matmul(   # denominators, pre-broadcast
                                    sm[64 * h:64 * h + 64, c0:512],
                                    ones_bf[:],
                                    pt[:, h * 512 + c0:(h + 1) * 512],
                                    start=first, stop=last,
                                    tile_position=(0, 64 * h),
                                )
                        rc = rcp.tile([128, 512], F32)
                        nc.vector.reciprocal(rc[:], sm[:])
                        nc.vector.tensor_mul(
                            attnT[:, pair * S + g * 512:pair * S + (g + 1) * 512],
                            av[:], rc[:],
                        )

            # ---- phase D: output projection (partial; host adds b_proj) ----
            with tc.tile_pool(name="yps", bufs=3, space="PSUM") as yps, \
                 tc.tile_pool(name="ystage", bufs=3) as ystage:
                for t in range(ST):
                    ps = yps.tile([128, N_EMBD], F32)
                    for p in range(N_PAIRS):
                        for h0, hw in ((0, 512), (512, 256)):
                            nc.tensor.matmul(
                                ps[:, h0:h0 + hw],
                                attnT[:, p * S + t * 128:p * S + (t + 1) * 128],
                                w_proj[:, p * N_EMBD + h0:p * N_EMBD + h0 + hw],
                                start=(p == 0), stop=(p == N_PAIRS - 1),
                            )
                    ys = ystage.tile([128, N_EMBD], F32)
                    nc.vector.tensor_copy(ys[:], ps[:])
                    nc.sync.dma_start(out=y_d[t * 128:(t + 1) * 128, :], in_=ys[:])

        if loop_n is not None:
            nc.sync.dma_start(out=tout_d[:], in_=dummy_d[:])

    nc.compile()
    return nc


def _numpy_fallback(x, mask, W_attn, b_attn, W_proj, b_proj):
    qkv = x @ W_attn + b_attn
    q, k, v = np.split(qkv, 3, axis=-1)

    def heads(t):
        return t.reshape(B, S, N_HEAD, HEAD_DIM).transpose(0, 2, 1, 3)

    q, k, v = heads(q), heads(k), heads(v)
    attn = np.einsum("bhqd,bhkd->bhqk", q, k) / np.sqrt(np.float32(HEAD_DIM))
    attn = attn + mask * (-1e9)
    attn = attn - attn.max(axis=-1, keepdims=True)
    attn = np.exp(attn)
    attn = attn / attn.sum(axis=-1, keepdims=True)
    out = np.einsum("bhqk,bhkd->bhqd", attn, v)
    out = out.transpose(0, 2, 1, 3).reshape(B, S, N_EMBD)
    return (out @ W_proj + b_proj).astype(np.float32)


def kernel(x, mask, W_attn, b_attn, W_proj, b_proj):
    global LAST_RESULTS, _PROGRAM
    x = np.asarray(x, dtype=np.float32)
    mask = np.asarray(mask, dtype=np.float32)
    W_attn = np.asarray(W_attn, dtype=np.float32)
    b_attn = np.asarray(b_attn, dtype=np.float32)
    W_proj = np.asarray(W_proj, dtype=np.float32)
    b_proj = np.asarray(b_proj, dtype=np.float32)

    # the kernel exploits causal structure; verify the mask actually is causal
    causal = 1.0 - np.tril(np.ones((S, S), dtype=np.float32))
    if mask.shape != (1, 1, S, S) or not np.array_equal(mask[0, 0], causal):
        return _numpy_fallback(x, mask, W_attn, b_attn, W_proj, b_proj)

    from concourse.bass_utils import run_bass_kernel_spmd

    if _PROGRAM is None:
        _PROGRAM = _build_program()

    in_maps = make_in_maps(x, W_attn, b_attn, W_proj)

    trace = bool(int(os.environ.get("ATTN_KERNEL_TRACE", "0")))
    res = run_bass_kernel_spmd(_PROGRAM, in_maps, list(range(N_CORES)), trace=trace)
    LAST_RESULTS = res

    y = np.zeros((B, S, N_EMBD), dtype=np.float32)
    for c in range(N_CORES):
        y[c // 2] += res.results[c]["y"]
    y += b_proj
    return y


def make_in_maps(x, W_attn, b_attn, W_proj):
    in_maps = []
    for c in range(N_CORES):
        b, hg = divmod(c, 2)
        o = HG_DIM * hg
        in_maps.append({
            "x": np.ascontiguousarray(x[b]),
            "w_qkv": np.ascontiguousarray(np.concatenate(
                [W_attn[:, o:o + HG_DIM],
                 W_attn[:, 768 + o:768 + o + HG_DIM],
                 W_attn[:, 1536 + o:1536 + o + HG_DIM]], axis=1)),
            "b_qk": np.ascontiguousarray(np.concatenate(
                [b_attn[o:o + HG_DIM], b_attn[768 + o:768 + o + HG_DIM]])),
            "b_v": np.ascontiguousarray(b_attn[1536 + o:1536 + o + HG_DIM]),
            "w_proj": np.ascontiguousarray(W_proj[o:o + HG_DIM, :]),
            "ones": np.ones((1, 128), dtype=np.float32),
            "ones_bf": np.ones((128, 64), dtype=ml_dtypes.bfloat16),
        })
    return in_maps
